# revision 28
# baseline (speedup 1.0000x reference)
"""ALiBi attention (B=2, N=2048, C=1024, H=16, D=64) on 8 TRN2 NeuronCores.

Sharding: core i owns heads (2i, 2i+1) for both batches (4 [N,N] score blocks
per core). Q/K/V/first-proj are column-split over heads; output projection is
computed n-sharded after head-split AllToAlls of the per-head attention
outputs (the head-0 AllToAll + half the output projection overlap the head-1
attention compute).

Precision: the reference DIVIDES by scale (multiplies scores by sqrt(D)=8), so
score noise from bf16 rounding of Q/K would be ~0.2 absolute. All matmuls that
feed scores therefore use an exact bf16 hi/lo split: main = hi*hi (exact in the
PE's fp32 accumulator) plus one stacked cross-term matmul (hi*lo + lo*hi).

Layouts (transposed activations, contraction on partitions):
  xT [C, B*N] -> qT/kT [e, n] per head; v natural [m, e].
  pass1 (row-max for softmax stability): S1[n, m] via lhsT=Q rhs=K, DVE
    reduce_max -> -M[n], folded back into Q's aug row via PE transpose + DMA.
  pass2: S2[m, n] = qk - slope*n - M[n] + bf16(slope*m) via aug rows; ACT exp
    adds the fp32 residual of slope*m as per-partition bias; the AV matmul
    carries a ones-column in V so the softmax denominator falls out free.
  Q/K aug tiles are zero-padded to K=128: K<=66 matmuls keep the PE HAM
  throttled at 1.2 GHz, K=128 runs 2x faster for the same column count.
"""
import numpy as np
import ml_dtypes

import concourse.bacc as bacc
import concourse.mybir as mybir
import concourse.tile as tile
from concourse.bass_utils import run_bass_kernel_spmd

F32 = mybir.dt.float32
BF16 = mybir.dt.bfloat16
BF = ml_dtypes.bfloat16

B, N, C, H, D = 2, 2048, 1024, 16, 64
NCORES = 8
HL = H // NCORES          # heads per core (2)
BN = B * N                # 4096
NSH = BN // NCORES        # 512 output columns per core
CCH = C // 128            # 8 contraction chunks
NBH = B * HL              # 4 (batch, local-head) blocks per core
MC = N // 128             # 16 m-chunks per sequence
AX = mybir.AxisListType
ALU = mybir.AluOpType
ACT = mybir.ActivationFunctionType

_compiled = None


def _build():
    nc = bacc.Bacc("TRN2", target_bir_lowering=False, debug=False,
                   num_devices=NCORES)

    x_hi = nc.dram_tensor("x_hi", [128, CCH, BN], BF16, kind="ExternalInput")
    x_lo = nc.dram_tensor("x_lo", [128, CCH, BN], BF16, kind="ExternalInput")
    wq_hi = nc.dram_tensor("wq_hi", [128, CCH, 128], BF16, kind="ExternalInput")
    wq_lo = nc.dram_tensor("wq_lo", [128, CCH, 128], BF16, kind="ExternalInput")
    wk_hi = nc.dram_tensor("wk_hi", [128, CCH, 128], BF16, kind="ExternalInput")
    wk_lo = nc.dram_tensor("wk_lo", [128, CCH, 128], BF16, kind="ExternalInput")
    wv = nc.dram_tensor("wv", [128, CCH, 128], BF16, kind="ExternalInput")
    wp = nc.dram_tensor("wp", [128, CCH, C], BF16, kind="ExternalInput")
    bp_t = nc.dram_tensor("bp_t", [128, CCH], F32, kind="ExternalInput")
    qaug = nc.dram_tensor("qaug", [HL, 3, N], BF16, kind="ExternalInput")
    kaug = nc.dram_tensor("kaug", [HL, 3, N], BF16, kind="ExternalInput")
    mbias = nc.dram_tensor("mbias", [128, HL * MC], F32, kind="ExternalInput")
    ident = nc.dram_tensor("ident", [128, 128], F32, kind="ExternalInput")
    out_t = nc.dram_tensor("out", [C, NSH], F32, kind="ExternalOutput")

    with tile.TileContext(nc) as tc:
        with tc.tile_pool(name="wpool", bufs=1) as wpool, \
             tc.tile_pool(name="xpool", bufs=1) as xpool, \
             tc.tile_pool(name="qkpool", bufs=1) as qkpool, \
             tc.tile_pool(name="aux", bufs=2) as aux, \
             tc.tile_pool(name="attp", bufs=1) as attp, \
             tc.tile_pool(name="psum", bufs=1, space="PSUM") as psum, \
             tc.tile_pool(name="dram", bufs=1, space="DRAM") as dram:

            # ---------- resident weights / aux ----------
            wq_hi_sb = wpool.tile([128, CCH, 128], BF16)
            wq_lo_sb = wpool.tile([128, CCH, 128], BF16)
            wk_hi_sb = wpool.tile([128, CCH, 128], BF16)
            wk_lo_sb = wpool.tile([128, CCH, 128], BF16)
            wv_sb = wpool.tile([128, CCH, 128], BF16)
            wp_sb = wpool.tile([128, CCH, C], BF16)
            bp_sb = wpool.tile([128, CCH], F32)
            mbias_sb = wpool.tile([128, HL * MC], F32)
            ident_sb = wpool.tile([128, 128], F32)
            for sb_t, dr_t in ((wq_hi_sb, wq_hi), (wk_hi_sb, wk_hi),
                               (wv_sb, wv)):
                nc.sync.dma_start(sb_t[:], dr_t[:, :])
            for sb_t, dr_t in ((wq_lo_sb, wq_lo), (wk_lo_sb, wk_lo)):
                nc.scalar.dma_start(sb_t[:], dr_t[:, :])
            # not needed until attention / output projection: off the hot path
            for sb_t, dr_t in ((ident_sb, ident), (mbias_sb, mbias),
                               (bp_sb, bp_t), (wp_sb, wp)):
                nc.gpsimd.dma_start(sb_t[:], dr_t[:, :])

            # ---------- per-(batch, local-head) persistent tiles ----------
            QT, KT, QC, KC, VA, MP = [], [], [], [], [], []
            for i in range(NBH):
                j = i % HL
                # rows 0-63 hi part; 64-66 aug rows; 67-127 zero (K=128 pad)
                q = qkpool.tile([128, N], BF16, name=f"Qt{i}", tag=f"Qt{i}")
                k = qkpool.tile([128, N], BF16, name=f"Kt{i}", tag=f"Kt{i}")
                qc = qkpool.tile([128, N], BF16, name=f"Qc{i}", tag=f"Qc{i}")
                kc = qkpool.tile([128, N], BF16, name=f"Kc{i}", tag=f"Kc{i}")
                va = qkpool.tile([128, MC, 65], BF16, name=f"Va{i}", tag=f"Va{i}")
                mp = qkpool.tile([128, 32], F32, name=f"Mp{i}", tag=f"Mp{i}")
                nc.any.memset(q[64:128, :], 0.0)
                nc.any.memset(k[64:128, :], 0.0)
                # q rows 64-66: [-slope*n; -M placeholder (0); ones]
                nc.sync.dma_start(q[64:67, :], qaug[j, :, :])
                # k rows 64-66: [ones; ones; bf16(slope*m)]
                nc.sync.dma_start(k[64:67, :], kaug[j, :, :])
                nc.any.memset(va[:, :, 64:65], 1.0)
                QT.append(q); KT.append(k); QC.append(qc); KC.append(kc)
                VA.append(va); MP.append(mp)

            # ---------- projections (4 block-pairs of 1024 over B*N) ----------
            def proj_pair(bp_i):
                b = bp_i // 2
                nw = bp_i % 2         # 1024-block within batch
                col0 = bp_i * 1024
                xh, xl = [], []
                for c in range(CCH):
                    th = xpool.tile([128, 1024], BF16, name=f"xh{bp_i}_{c}",
                                    tag="xh", bufs=12)
                    tl = xpool.tile([128, 1024], BF16, name=f"xl{bp_i}_{c}",
                                    tag="xl", bufs=12)
                    nc.sync.dma_start(th[:], x_hi[:, c, col0:col0 + 1024])
                    nc.sync.dma_start(tl[:], x_lo[:, c, col0:col0 + 1024])
                    xh.append(th); xl.append(tl)

                cols = slice(nw * 1024, nw * 1024 + 1024)
                for w_hi_t, w_lo_t, T, TC, is_q in (
                        (wq_hi_sb, wq_lo_sb, QT, QC, True),
                        (wk_hi_sb, wk_lo_sb, KT, KC, False)):
                    ps = psum.tile([128, 1024], F32, name=f"pj{bp_i}_{int(is_q)}",
                                   tag="score", bufs=2)
                    for half in range(2):
                        hs = slice(half * 512, half * 512 + 512)
                        nmm = 3 * CCH
                        idx = 0
                        for c in range(CCH):
                            nc.tensor.matmul(ps[:, hs], w_hi_t[:, c, :],
                                             xh[c][:, hs],
                                             start=(idx == 0), stop=(idx == nmm - 1))
                            idx += 1
                        for c in range(CCH):
                            nc.tensor.matmul(ps[:, hs], w_lo_t[:, c, :],
                                             xh[c][:, hs],
                                             start=False, stop=(idx == nmm - 1))
                            idx += 1
                        for c in range(CCH):
                            nc.tensor.matmul(ps[:, hs], w_hi_t[:, c, :],
                                             xl[c][:, hs],
                                             start=False, stop=(idx == nmm - 1))
                            idx += 1
                        yield
                    for j in range(HL):
                        i = b * HL + j
                        rows = slice(64 * j, 64 * j + 64)
                        nc.any.tensor_copy(T[i][0:64, cols], ps[rows, :])
                        if is_q:   # Qc = [q_hi; q_lo]
                            nc.any.tensor_copy(TC[i][0:64, cols], T[i][0:64, cols])
                            nc.vector.tensor_sub(TC[i][64:128, cols], ps[rows, :],
                                                 T[i][0:64, cols])
                        else:      # Kc = [k_lo; k_hi]
                            nc.any.tensor_copy(TC[i][64:128, cols], T[i][0:64, cols])
                            nc.vector.tensor_sub(TC[i][0:64, cols], ps[rows, :],
                                                 T[i][0:64, cols])
                    yield

                # v in natural [m, e] layout
                for mt in range(8):
                    vps = psum.tile([128, 128], F32, name=f"v{bp_i}_{mt}",
                                    tag="avsm", bufs=2)
                    for c in range(CCH):
                        nc.tensor.matmul(vps[:], xh[c][:, mt * 128:(mt + 1) * 128],
                                         wv_sb[:, c, :],
                                         start=(c == 0), stop=(c == CCH - 1))
                    mc = nw * 8 + mt
                    for j in range(HL):
                        i = b * HL + j
                        nc.any.tensor_copy(VA[i][:, mc, 0:64],
                                           vps[:, 64 * j:64 * j + 64])
                    if mt % 2 == 1:
                        yield

            # ---------- attention ----------
            # head-split AllToAll buffers: ag[j] carries local head j's rows
            ag_in = [dram.tile([NCORES, 64, NSH], BF16, name=f"agi{j}")
                     for j in range(HL)]
            ag_out = [dram.tile([NCORES, 64, NSH], BF16, name=f"ago{j}")
                      for j in range(HL)]

            def pass1(i):
                Q, K, Mpt = QT[i], KT[i], MP[i]
                for nt in range(16):
                    for half in range(2):
                        ps = psum.tile([128, 1024], F32, tag="score", bufs=2,
                                       name=f"p1_{i}_{nt}_{half}")
                        for mb in range(2):
                            m0 = (half * 2 + mb) * 512
                            nc.tensor.matmul(ps[:, mb * 512:(mb + 1) * 512],
                                             Q[:, nt * 128:(nt + 1) * 128],
                                             K[:, m0:m0 + 512],
                                             start=True, stop=True)
                        nc.vector.tensor_reduce(
                            Mpt[:, nt * 2 + half:nt * 2 + half + 1], ps[:, :],
                            axis=AX.X, op=ALU.max)
                        yield
                mneg = aux.tile([128, 16], F32, tag="mneg", name=f"mneg{i}")
                nc.vector.tensor_reduce(
                    mneg[:], Mpt[:].rearrange("p (a b) -> p a b", b=2),
                    axis=AX.X, op=ALU.max, negate=True)
                trp = psum.tile([16, 128], F32, tag="avsm", bufs=2, name=f"trp{i}")
                nc.tensor.transpose(trp[:], mneg[:], ident_sb[:])
                mrow16 = aux.tile([16, 128], BF16, tag="mrow16", name=f"mr{i}")
                nc.any.tensor_copy(mrow16[:], trp[:])
                nc.gpsimd.dma_start(QT[i][65:66, :], mrow16[:, :])
                yield

            def pass2(i):
                b, j = divmod(i, HL)
                Q, K, Qc, Kc, Va = QT[i], KT[i], QC[i], KC[i], VA[i]
                for nb in range(2):
                    n0 = nb * 1024
                    avp = psum.tile([65, 1024], F32, tag="avsm", bufs=2,
                                    name=f"av_{i}_{nb}")
                    at_q = []

                    def emit_av(mc, at):
                        for hf in range(2):
                            hs = slice(hf * 512, hf * 512 + 512)
                            nc.tensor.matmul(avp[:, hs], Va[:, mc, :], at[:, hs],
                                             start=(mc == 0), stop=(mc == MC - 1))

                    for mc in range(MC):
                        s2 = psum.tile([128, 1024], F32, tag="score", bufs=2,
                                       name=f"s2_{i}_{nb}_{mc}")
                        for hf in range(2):
                            hs = slice(hf * 512, hf * 512 + 512)
                            ns = slice(n0 + hf * 512, n0 + hf * 512 + 512)
                            nc.tensor.matmul(s2[:, hs],
                                             K[:, mc * 128:(mc + 1) * 128],
                                             Q[:, ns], start=True, stop=False)
                            nc.tensor.matmul(s2[:, hs],
                                             Kc[:, mc * 128:(mc + 1) * 128],
                                             Qc[:, ns], start=False, stop=True)
                        at = attp.tile([128, 1024], BF16, tag="att", bufs=4,
                                       name=f"at_{i}_{nb}_{mc}")
                        nc.scalar.activation(at[:], s2[:], ACT.Exp,
                                             bias=mbias_sb[:, j * MC + mc:j * MC + mc + 1],
                                             scale=1.0)
                        # av for the PREVIOUS chunk: its exp has had a full
                        # s2-round to drain, so the PE never waits on ACT
                        at_q.append((mc, at))
                        if len(at_q) > 1:
                            emit_av(*at_q.pop(0))
                        yield
                    emit_av(*at_q.pop(0))
                    # normalize: reciprocal spread over 32 partitions (DVE
                    # reciprocal is ~8 cyc/elem/lane)
                    lrow = aux.tile([1, 1024], F32, tag="lrow", bufs=2, name=f"lr_{i}_{nb}")
                    nc.any.tensor_copy(lrow[0:1, :], avp[64:65, :])
                    l32 = aux.tile([32, 32], F32, tag="l32", bufs=2, name=f"l32_{i}_{nb}")
                    nc.gpsimd.dma_start(l32[:, :], lrow[0:1, :])
                    r32 = aux.tile([32, 32], F32, tag="r32", bufs=2, name=f"r32_{i}_{nb}")
                    nc.vector.reciprocal(r32[:], l32[:])
                    linv = aux.tile([1, 1024], F32, tag="linv", bufs=2, name=f"li_{i}_{nb}")
                    nc.gpsimd.dma_start(linv[0:1, :], r32[:, :])
                    lb = aux.tile([64, 1024], F32, tag="lb", bufs=2, name=f"lb_{i}_{nb}")
                    nc.gpsimd.partition_broadcast(lb[:], linv[0:1, :])
                    gt = aux.tile([64, 1024], BF16, tag="gt", bufs=2, name=f"gt_{i}_{nb}")
                    nc.vector.tensor_mul(gt[:], avp[0:64, :], lb[:])
                    for hf in range(2):
                        s = b * 4 + nb * 2 + hf
                        nc.sync.dma_start(ag_in[j][s, :, :],
                                          gt[:, hf * 512:hf * 512 + 512])
                    yield

            gt_in = attp.tile([128, CCH, NSH], BF16, tag="gtin", bufs=1)

            def emit_a2a(j):
                nc.gpsimd.collective_compute(
                    "AllToAll", ALU.bypass,
                    replica_groups=[list(range(NCORES))],
                    ins=[ag_in[j].opt()],
                    outs=[ag_out[j].opt()],
                )
                rows = slice(64 * j, 64 * j + 64)
                engs = [nc.sync, nc.gpsimd]
                for c in range(CCH):
                    engs[c % 2].dma_start(gt_in[rows, c, :], ag_out[j][c, :, :])

            def drive(*gens_weights):
                """Round-robin generators with weights until all exhausted."""
                gens = [[g, w] for g, w in gens_weights]
                while gens:
                    for gw in list(gens):
                        g, w = gw
                        for _ in range(w):
                            try:
                                next(g)
                            except StopIteration:
                                gens.remove(gw)
                                break

            def chain(*gens):
                for g in gens:
                    yield from g

            # batch-0 proj; then batch-1 proj interleaved with head-0 pass1;
            # then pass2(i) interleaved with the next pass1
            drive((chain(proj_pair(0), proj_pair(1)), 1))
            drive((chain(proj_pair(2), proj_pair(3)), 1),
                  (pass1(0), 3))
            drive((pass2(0), 1), (chain(pass1(2), pass1(1)), 2))
            drive((pass2(2), 1), (pass1(3), 1))
            emit_a2a(0)
            drive((pass2(1), 1))
            drive((pass2(3), 1))
            emit_a2a(1)

            # ---------- output projection ----------
            for et in range(CCH):
                yps = psum.tile([128, 512], F32, tag="avsm", bufs=2, name=f"y{et}")
                for c in range(CCH):
                    nc.tensor.matmul(yps[:], wp_sb[:, c, et * 128:(et + 1) * 128],
                                     gt_in[:, c, :],
                                     start=(c == 0), stop=(c == CCH - 1))
                ysb = aux.tile([128, 512], F32, tag="y", name=f"ysb{et}")
                nc.scalar.activation(ysb[:], yps[:], ACT.Identity,
                                     bias=bp_sb[:, et:et + 1], scale=1.0)
                nc.sync.dma_start(out_t[et * 128:(et + 1) * 128, :], ysb[:])

    nc.compile()
    return nc


def _get_nc():
    global _compiled
    if _compiled is None:
        _compiled = _build()
    return _compiled


def _alibi_slopes():
    x = (2 ** 8) ** (1.0 / H)
    return np.array([1.0 / x ** (i + 1) for i in range(H)], dtype=np.float64)


def _chunked(a):
    """[C, F] -> [128, CCH, F] (partition, c-chunk, free)."""
    Cdim, F = a.shape
    return np.ascontiguousarray(a.reshape(CCH, 128, F).transpose(1, 0, 2))


def _split(a):
    hi = a.astype(BF)
    lo = (a - hi.astype(np.float32)).astype(BF)
    return hi, lo


def _make_in_maps(x, Wq, Wk, Wv, Wp, bp):
    x = np.asarray(x, dtype=np.float32)
    xT = np.ascontiguousarray(x.reshape(BN, C).T)          # [C, BN]
    xch = _chunked(xT)
    xch_hi, xch_lo = _split(xch)

    slopes = _alibi_slopes()
    n_arr = np.arange(N, dtype=np.float64)
    p_arr = np.arange(128, dtype=np.float64)

    wp_ch = _chunked(np.ascontiguousarray(np.asarray(Wp, np.float32).T)).astype(BF)
    bp_tile = np.ascontiguousarray(
        np.asarray(bp, np.float32).reshape(CCH, 128).T)
    identity = np.eye(128, dtype=np.float32)

    in_maps = []
    for core in range(NCORES):
        e0 = core * 128
        wqT = np.ascontiguousarray((8.0 * np.asarray(Wq, np.float32)[e0:e0 + 128]).T)
        wkT = np.ascontiguousarray(np.asarray(Wk, np.float32)[e0:e0 + 128].T)
        wvT = np.ascontiguousarray(np.asarray(Wv, np.float32)[e0:e0 + 128].T)
        wq_h, wq_l = _split(_chunked(wqT))
        wk_h, wk_l = _split(_chunked(wkT))

        s = slopes[core * HL: core * HL + HL]               # [HL]
        qa = np.zeros((HL, 3, N), dtype=BF)
        ka = np.zeros((HL, 3, N), dtype=BF)
        # pass2's K=128 main matmul already adds bf16(slope*m) via k row 66;
        # the exp bias supplies only the fp32 residual so the total is exact
        mb = np.zeros((128, HL * MC), dtype=np.float32)
        for j in range(HL):
            qa[j, 0] = (-s[j] * n_arr).astype(BF)   # -slope*n
            qa[j, 1] = 0.0                          # -M placeholder
            qa[j, 2] = 1.0
            ka[j, 0] = 1.0
            ka[j, 1] = 1.0
            ka[j, 2] = (s[j] * n_arr).astype(BF)    # bf16(slope*m)
            for c in range(MC):
                exact = (s[j] * (128 * c + p_arr)).astype(np.float32)
                mb[:, j * MC + c] = exact - exact.astype(BF).astype(np.float32)

        in_maps.append({
            "x_hi": xch_hi, "x_lo": xch_lo,
            "wq_hi": wq_h, "wq_lo": wq_l,
            "wk_hi": wk_h, "wk_lo": wk_l,
            "wv": _chunked(wvT).astype(BF),
            "wp": wp_ch, "bp_t": bp_tile,
            "qaug": qa, "kaug": ka, "mbias": mb,
            "ident": identity,
        })
    return in_maps


def run(x, Wq, Wk, Wv, Wp, bp, trace=False, tmpdir=None):
    nc = _get_nc()
    in_maps = _make_in_maps(x, Wq, Wk, Wv, Wp, bp)
    kwargs = {}
    if trace:
        kwargs = {"trace": True, "tmpdir": tmpdir}
    res = run_bass_kernel_spmd(nc, in_maps, core_ids=list(range(NCORES)), **kwargs)
    yT = np.concatenate([res.results[i]["out"] for i in range(NCORES)], axis=1)
    out = np.ascontiguousarray(yT.T).reshape(B, N, C).astype(np.float32)
    return out, res


def kernel(x, Wq, Wk, Wv, Wp, bp):
    out, _ = run(x, Wq, Wk, Wv, Wp, bp)
    return out
